# revision 1
# baseline (speedup 1.0000x reference)
"""GCN (6-layer: conv1 + 4x shared conv2 + mean-pool + linear) on 8 Trainium2
NeuronCores via Bass/Tile.

Strategy (dst-sharded message passing with a replicated gather table):
  - Nodes are sharded contiguously across cores (NPC = N/C per core).
  - Per conv layer: every core transforms its own nodes (h @ W), scales rows
    by dis = deg^-1/2 (GCN norm factorization: enorm = dis[src]*dis[dst], so
    agg[d] = dis[d] * sum_e t[src_e] with t = dis*hw, self term = dis[d]*t[d]),
    and the per-core row slices are AllGather'd into a replicated HBM table.
  - Each core gathers its incident edges' source rows (dma_gather, 256B rows)
    and segment-sums them by destination via one-hot matmuls on the
    TensorEngine (S built on-chip with an iota/is_equal compare, PSUM
    accumulation per 128-node window).
  - Mean-pool partial sums per graph are computed with the same one-hot
    matmul trick, AllGather'd (small), and every core computes the identical
    final linear readout; core 0's output is returned.

The single SPMD program is identical on all cores: all per-core variation
travels through input tensors; chunk/bucket counts are padded to the max
over cores so the instruction stream is uniform.
"""

import math
import sys

sys.path.insert(0, "/opt/trn_rl_repo")

import numpy as np
import ml_dtypes

import concourse.bass as bass
import concourse.mybir as mybir
import concourse.tile as tile
from concourse import bacc
from concourse.masks import make_identity

BF16 = mybir.dt.bfloat16
F32 = mybir.dt.float32
I16 = mybir.dt.int16
ALU = mybir.AluOpType

NP_BF16 = ml_dtypes.bfloat16

CALLCH = 8         # chunks per dma_gather call (1024 idxs = HW packet limit)
SBATCH = 8         # chunks per S-build DVE op
WGRP = 16          # dst windows per PSUM accumulation group


def _ap3(ap, pattern, offset=None):
    """Hand-build a broadcast/strided AP on the same tensor."""
    return bass.AP(ap.tensor, ap.offset if offset is None else offset, pattern)


# ---------------------------------------------------------------------------
# Host preprocessing
# ---------------------------------------------------------------------------

def prep(x, W1, b1, W2, b2, Wl, bl, edge_index, batch, C, G):
    x = np.asarray(x, np.float32)
    W1 = np.asarray(W1, np.float32); b1 = np.asarray(b1, np.float32)
    W2 = np.asarray(W2, np.float32); b2 = np.asarray(b2, np.float32)
    Wl = np.asarray(Wl, np.float32); bl = np.asarray(bl, np.float32)
    edge_index = np.asarray(edge_index, np.int64)
    batch = np.asarray(batch, np.int64)

    N, F = x.shape
    E = edge_index.shape[1]
    H = W1.shape[1]
    assert N % C == 0 and C % 2 == 0
    NPC = N // C
    W = math.ceil(NPC / 128)
    NPAD = W * 128
    ROWS = C * NPAD
    HR = (C // 2) * NPAD
    assert HR <= 32768, HR
    NG = math.ceil(W / WGRP)

    src, dst = edge_index[0], edge_index[1]
    deg = 1.0 + np.bincount(dst, minlength=N).astype(np.float32)
    dis = 1.0 / np.sqrt(deg)

    n = np.arange(N)
    cb = n // NPC
    lp = n % NPC
    p_ = lp % 128
    w_ = lp // 128
    srow = cb * NPAD + p_ * W + w_      # p-major table row of node n
    xcol = cb * NPAD + w_ * 128 + p_    # window-major x_fm column of node n

    # --- edge bucketing -----------------------------------------------------
    ecore = dst // NPC
    edl = dst % NPC
    ew = edl // 128
    ewp = edl // 256            # window-pair (chunks span 2 windows)
    edloc = edl % 256           # dst-local within the pair
    esh = (srow[src] >= HR).astype(np.int64)
    WP = math.ceil(W / 2)
    PPG = WGRP // 2             # pairs per psum group

    cnt = np.zeros((C, 2, WP), np.int64)
    np.add.at(cnt, (ecore, esh, ewp), 1)
    Kb = np.ceil(cnt.max(axis=0) / 128).astype(np.int64)   # [2, WP] chunks
    for wp in range(WP):
        if Kb[:, wp].sum() == 0:
            Kb[0, wp] = 1

    # chunk order: (group, src-half, window-pair)
    chunk_w, chunk_sh = [], []
    seg_bounds = []   # (sh, lo, hi) per (g, sh) segment
    boff = np.zeros((2, WP), np.int64)  # first chunk index of bucket (sh, wp)
    for g in range(NG):
        plo, phi = g * PPG, min((g + 1) * PPG, WP)
        for sh in (0, 1):
            lo = len(chunk_w)
            for wp in range(plo, phi):
                boff[sh, wp] = len(chunk_w)
                for _ in range(int(Kb[sh, wp])):
                    chunk_w.append(wp); chunk_sh.append(sh)
            if len(chunk_w) > lo:
                seg_bounds.append((sh, lo, len(chunk_w)))
    NCHUNK = len(chunk_w)
    chunk_w = np.array(chunk_w); chunk_sh = np.array(chunk_sh)

    # start/stop flags: first/last chunk per PSUM BANK (8 windows of 64 f32
    # = one 2KB zero region; the start bit lazily zeroes the whole bank).
    # Both windows of a pair are always in the same bank.
    bank_of_chunk = (2 * chunk_w) // 8
    start_f = np.zeros(NCHUNK, bool); stop_f = np.zeros(NCHUNK, bool)
    for b in np.unique(bank_of_chunk):
        idxs = np.nonzero(bank_of_chunk == b)[0]
        start_f[idxs[0]] = True; stop_f[idxs[-1]] = True

    # gather calls: slice each segment into <= CALLCH chunks
    calls = []
    for sh, lo, hi in seg_bounds:
        c0 = lo
        while c0 < hi:
            c1 = min(c0 + CALLCH, hi)
            calls.append((sh, c0, c1))
            c0 = c1
    grp_of_chunk = (2 * chunk_w) // WGRP

    # --- per-core edge payloads --------------------------------------------
    idx_all = np.zeros((C, NCHUNK * 128), np.int16)
    dl_all = np.full((C, NCHUNK * 128), 400.0, np.float32)
    for c in range(C):
        m = ecore == c
        es, ish, iw, idl = src[m], esh[m], ewp[m], edloc[m]
        order = np.lexsort((iw, ish))
        es, ish, iw, idl = es[order], ish[order], iw[order], idl[order]
        # rank within bucket
        key = ish * WP + iw
        # edges sorted by key; position = boff[bucket]*128 + rank-in-bucket
        uniq, first = np.unique(key, return_index=True)
        ranks = np.arange(len(key)) - first[np.searchsorted(uniq, key)]
        pos = boff[ish, iw] * 128 + ranks
        idx_all[c, pos] = (srow[es] - ish * HR).astype(np.int16)
        dl_all[c, pos] = idl

    # wrapped-16 index layout, replicated to 128 partitions
    idx16 = np.zeros((C, 128, NCHUNK * 8), np.int16)
    for c in range(C):
        wrapped = idx_all[c].reshape(NCHUNK * 8, 16).T   # [16, NCHUNK*8]
        idx16[c] = np.tile(wrapped, (8, 1))
    dstloc = np.zeros((C, 128, NCHUNK), NP_BF16)
    for c in range(C):
        dstloc[c] = dl_all[c].reshape(NCHUNK, 128).T.astype(NP_BF16)

    # --- node-side tensors --------------------------------------------------
    xfm = np.zeros((F, ROWS), np.float32)
    xfm[:, xcol] = x.T
    xfm = xfm.astype(NP_BF16)

    disALL = np.zeros((128, C * W), np.float32)
    disALL[p_, cb * W + w_] = dis
    disn = np.zeros((C, 128, W), np.float32)
    for c in range(C):
        sl = slice(c * NPC, (c + 1) * NPC)
        disn[c][p_[sl], w_[sl]] = dis[sl]

    tailp = NPC - (W - 1) * 128
    mask48 = (np.arange(128) < tailp).astype(np.float32).reshape(128, 1)

    # --- pooling ------------------------------------------------------------
    BLK = math.ceil(G / 128) + 3
    wk = [int(batch[c * NPC]) // 128 for c in range(C)]
    glocal = np.full((C, 128, W), 1.0e4, np.float32)
    for c in range(C):
        sl = slice(c * NPC, (c + 1) * NPC)
        gl = batch[sl] - 128 * wk[c]
        assert gl.min() >= 0 and gl.max() < 384, (c, gl.min(), gl.max())
        glocal[c][p_[sl], w_[sl]] = gl
    counts = np.bincount(batch, minlength=G).astype(np.float32)
    invc = np.ones((128, BLK), np.float32)
    gg = np.arange(G)
    invc[gg % 128, gg // 128] = 1.0 / np.maximum(counts, 1.0)

    meta = dict(
        N=N, F=F, H=H, E=E, G=G, C=C, NPC=NPC, W=W, NPAD=NPAD, ROWS=ROWS,
        HR=HR, NG=NG, NCHUNK=NCHUNK, chunk_w=chunk_w, chunk_sh=chunk_sh,
        bank_of_chunk=bank_of_chunk,
        start_f=start_f, stop_f=stop_f, calls=calls, grp_of_chunk=grp_of_chunk,
        wk=wk, BLK=BLK, blv=float(bl.reshape(-1)[0]), tailp=tailp,
    )

    shared = dict(
        xfm=xfm,
        disALL=disALL,
        W1sb=W1.astype(NP_BF16),
        W2sb=W2.astype(NP_BF16),
        Wlsb=Wl.astype(NP_BF16),
        b1sb=np.tile(b1.reshape(1, H), (128, 1)).astype(np.float32),
        b2sb=np.tile(b2.reshape(1, H), (128, 1)).astype(np.float32),
        iota128=np.tile(np.arange(256, dtype=np.float32).reshape(1, 256),
                        (128, 1)).astype(NP_BF16),
        iotaP=np.tile(np.arange(384, dtype=np.float32).reshape(1, 384),
                      (128, 1)),
        invc=invc,
        mask48=mask48,
    )
    in_maps = []
    for c in range(C):
        m = dict(shared)
        m["xown"] = np.ascontiguousarray(
            xfm[:, c * NPAD:(c + 1) * NPAD])
        m["idx16"] = idx16[c]
        m["dstloc"] = dstloc[c]
        m["disn"] = disn[c]
        m["glocal"] = glocal[c]
        in_maps.append(m)
    return meta, in_maps


# ---------------------------------------------------------------------------
# Bass/Tile program
# ---------------------------------------------------------------------------

def build(nc, meta, debug=False, stages=99):
    F, H, C = meta["F"], meta["H"], meta["C"]
    W, NPAD, ROWS, HR = meta["W"], meta["NPAD"], meta["ROWS"], meta["HR"]
    NG, NCHUNK, BLK, G = meta["NG"], meta["NCHUNK"], meta["BLK"], meta["G"]
    chunk_w, chunk_sh = meta["chunk_w"], meta["chunk_sh"]
    start_f, stop_f = meta["start_f"], meta["stop_f"]
    calls, wk = meta["calls"], meta["wk"]
    rg = [list(range(C))]

    # external inputs
    xfm_e = nc.dram_tensor("xfm", [F, ROWS], BF16, kind="ExternalInput")
    disALL_e = nc.dram_tensor("disALL", [128, C * W], F32, kind="ExternalInput")
    W1_e = nc.dram_tensor("W1sb", [F, H], BF16, kind="ExternalInput")
    W2_e = nc.dram_tensor("W2sb", [H, H], BF16, kind="ExternalInput")
    Wl_e = nc.dram_tensor("Wlsb", [H, 1], BF16, kind="ExternalInput")
    b1_e = nc.dram_tensor("b1sb", [128, H], F32, kind="ExternalInput")
    b2_e = nc.dram_tensor("b2sb", [128, H], F32, kind="ExternalInput")
    iota_e = nc.dram_tensor("iota128", [128, 256], BF16, kind="ExternalInput")
    iotaP_e = nc.dram_tensor("iotaP", [128, 384], F32, kind="ExternalInput")
    invc_e = nc.dram_tensor("invc", [128, BLK], F32, kind="ExternalInput")
    mask_e = nc.dram_tensor("mask48", [128, 1], F32, kind="ExternalInput")
    xown_e = nc.dram_tensor("xown", [F, NPAD], BF16, kind="ExternalInput")
    idx_e = nc.dram_tensor("idx16", [128, NCHUNK * 8], I16, kind="ExternalInput")
    dstloc_e = nc.dram_tensor("dstloc", [128, NCHUNK], BF16, kind="ExternalInput")
    disn_e = nc.dram_tensor("disn", [128, W], F32, kind="ExternalInput")
    glocal_e = nc.dram_tensor("glocal", [128, W], F32, kind="ExternalInput")
    out_e = nc.dram_tensor("out", [128, 3 * H], F32, kind="ExternalOutput")
    dbg = {}
    if debug:
        W_ = W; H_ = H
        dbg["tbl1"] = nc.dram_tensor("dbg_tbl1", [ROWS, 128], BF16,
                                     kind="ExternalOutput")
        dbg["town1"] = nc.dram_tensor("dbg_town1", [128, W_ * H_], BF16,
                                      kind="ExternalOutput")
        for l in range(1, 6):
            dbg[f"h{l}"] = nc.dram_tensor(f"dbg_h{l}", [128, W_ * H_], BF16,
                                          kind="ExternalOutput")
        dbg["agg1"] = nc.dram_tensor("dbg_agg1", [128, W_ * H_], F32,
                                     kind="ExternalOutput")
        dbg["msg1"] = nc.dram_tensor("dbg_msg1", [8, 128, CALLCH * 128], BF16,
                                     kind="ExternalOutput")
        dbg["S1"] = nc.dram_tensor("dbg_S1", [40, 128, SBATCH * 128], BF16,
                                   kind="ExternalOutput")
        dbg["poolf"] = nc.dram_tensor("dbg_poolf", [128, BLK * H_], F32,
                                      kind="ExternalOutput")

    # internal DRAM
    PW = 3 * H
    shared = "Shared" if C > 4 else "Local"
    tbl = nc.dram_tensor("tbl", [ROWS, 128], BF16, addr_space=shared)
    own_slice = nc.dram_tensor("own_slice", [NPAD, 128], BF16)
    ppart = nc.dram_tensor("ppart", [128, PW], BF16)
    pall = nc.dram_tensor("pall", [C, 128, PW], BF16, addr_space=shared)

    with tile.TileContext(nc) as tc:
        from contextlib import ExitStack
        with ExitStack() as ctx:
            cpool = ctx.enter_context(tc.tile_pool(name="const", bufs=1))
            spool = ctx.enter_context(tc.tile_pool(name="s", bufs=8))
            mpool = ctx.enter_context(tc.tile_pool(name="msg", bufs=8))
            hpool = ctx.enter_context(tc.tile_pool(name="h", bufs=1))
            tpool = ctx.enter_context(tc.tile_pool(name="tmp", bufs=3))
            agg_ps = ctx.enter_context(
                tc.tile_pool(name="aggps", bufs=2, space="PSUM"))
            mm_ps = ctx.enter_context(
                tc.tile_pool(name="mmps", bufs=2, space="PSUM"))
            tp_ps = ctx.enter_context(
                tc.tile_pool(name="tpps", bufs=2, space="PSUM"))

            # ---- load constants into SBUF ----
            def load(name, ext, shape, dt):
                t = cpool.tile(shape, dt, tag=name)
                nc.sync.dma_start(t[:], ext.ap())
                return t

            W1sb = load("W1", W1_e, [F, H], BF16)
            W2sb = load("W2", W2_e, [H, H], BF16)
            Wlsb = load("Wl", Wl_e, [H, 1], BF16)
            b1sb = load("b1", b1_e, [128, H], F32)
            b2sb = load("b2", b2_e, [128, H], F32)
            iota = load("iota", iota_e, [128, 256], BF16)
            iotaP = load("iotaP", iotaP_e, [128, 384], F32)
            invc = load("invc", invc_e, [128, BLK], F32)
            mask48 = load("mask48", mask_e, [128, 1], F32)
            idxsb = load("idx", idx_e, [128, NCHUNK * 8], I16)
            dstloc = load("dstloc", dstloc_e, [128, NCHUNK], BF16)
            disn = load("disn", disn_e, [128, W], F32)
            disALL = load("disALL", disALL_e, [128, C * W], F32)
            glocal = load("glocal", glocal_e, [128, W], F32)
            ident = cpool.tile([128, 128], BF16, tag="ident")
            make_identity(nc, ident[:])
            identF = cpool.tile([128, 128], F32, tag="identF")
            make_identity(nc, identF[:])

            # persistent per-layer state tiles
            h_fm = [cpool.tile([H, NPAD], BF16, tag=f"hfm{i}", name=f"hfm{i}")
                    for i in range(2)]
            t_own = cpool.tile([128, W * H], BF16, tag="town")
            tpad = cpool.tile([128, W * 128], BF16, tag="tpad")
            nc.vector.memset(tpad[:], 0.0)
            h_nm = cpool.tile([128, W * H], BF16, tag="hnm")

            def iota_bc(nb, width=256, base=0):
                a = iota[:]
                return _ap3(a, [[a.ap[0][0], 128], [0, nb], [1, width]], base)

            def sc_bc(t, lo, n, inner, pitch=None):
                a = t[:]
                step = a.ap[0][0]
                return _ap3(a, [[step, 128], [1, n], [0, inner]], a.offset + lo)

            # ---------------- conv transform helpers ----------------
            def transform_own_grp(h_src, Wsb, g):
                """own-node transform + t_own (dis*hw) + padded copy, windows
                of group g only."""
                wlo = g * WGRP
                nb = min(WGRP, W - wlo)
                ps = agg_ps.tile([128, WGRP * H], F32, tag="agg",
                                 name=f"tf{g}")
                for i in range(nb):
                    w = wlo + i
                    nc.tensor.matmul(
                        ps[:, i * H:(i + 1) * H],
                        lhsT=h_src[:, w * 128:(w + 1) * 128],
                        rhs=Wsb[:],
                        start=True, stop=True, skip_group_check=True)
                ps3 = _ap3(ps[:], [[ps[:].ap[0][0], 128], [H, nb], [1, H]])
                t3 = _ap3(t_own[:], [[t_own[:].ap[0][0], 128], [H, nb], [1, H]],
                          t_own[:].offset + wlo * H)
                nc.vector.tensor_tensor(
                    t3, ps3, sc_bc(disn, wlo, nb, H), op=ALU.mult)
                # spread compact -> padded (pad half is stale junk, never read)
                tp3 = _ap3(tpad[:], [[tpad[:].ap[0][0], 128], [128, nb], [1, H]],
                           tpad[:].offset + wlo * 128)
                to3 = _ap3(t_own[:], [[t_own[:].ap[0][0], 128], [H, nb], [1, H]],
                           t_own[:].offset + wlo * H)
                nc.vector.tensor_copy(tp3, to3)

            def transform_own(h_src, Wsb, K=None):
                for g in range(NG):
                    transform_own_grp(h_src, Wsb, g)

            def transform_all_conv1(xpool):
                """conv1: full-graph transform, writes the whole table."""
                NBAT = math.ceil(W / WGRP)
                for cblk in range(C):
                    xblk = xpool.tile([F, NPAD], BF16, tag="xfm")
                    nc.sync.dma_start(
                        xblk[:], xfm_e.ap()[:, cblk * NPAD:(cblk + 1) * NPAD])
                    for b in range(NBAT):
                        wlo = b * WGRP
                        nb = min(WGRP, W - wlo)
                        ps = agg_ps.tile([128, WGRP * H], F32, tag="agg")
                        for i in range(nb):
                            col = (wlo + i) * 128
                            nc.tensor.matmul(
                                ps[:, i * H:(i + 1) * H],
                                lhsT=xblk[:, col:col + 128],
                                rhs=W1sb[:],
                                start=True, stop=True, skip_group_check=True)
                        pd = tpool.tile([128, WGRP * 128], BF16, tag="c1pad")
                        nc.vector.memset(pd[:], 0.0)
                        ps3 = _ap3(ps[:], [[ps[:].ap[0][0], 128], [H, nb], [1, H]])
                        pd3 = _ap3(pd[:], [[pd[:].ap[0][0], 128], [128, nb], [1, H]])
                        nc.vector.tensor_tensor(
                            pd3, ps3, sc_bc(disALL, cblk * W + wlo, nb, H),
                            op=ALU.mult)
                        # rows cblk*NPAD + p*W + (wlo..wlo+nb)
                        dr = _ap3(tbl.ap(),
                                  [[W * 128, 128], [128, nb], [1, 128]],
                                  (cblk * NPAD + wlo) * 128)
                        nc.sync.dma_start(dr, pd[:, :nb * 128])

            def exchange():
                # padded own slice -> own_slice dram (contiguous), AllGather
                nc.sync.dma_start(
                    _ap3(own_slice.ap(), [[W * 128, 128], [1, W * 128]], 0),
                    tpad[:])
                nc.gpsimd.collective_compute(
                    "AllGather", ALU.bypass, replica_groups=rg,
                    ins=[own_slice.ap().opt()], outs=[tbl.ap().opt()])

            # ---------------- aggregation (+ per-group drain) ----------------
            def aggregate_and_drain(layer, bsb, after_group=None):
                ncalls = 0
                nsb = 0
                psg = {}
                bank_start = {}
                glast = {}
                for ci in range(NCHUNK):
                    glast[int(meta["grp_of_chunk"][ci])] = ci
                drain_after = {v: k for k, v in glast.items()}
                for (sh, c_lo, c_hi) in calls:
                    ncall = c_hi - c_lo
                    msg = mpool.tile([128, CALLCH * 128], BF16, tag="msg")
                    in_ap = bass.AP(tbl.ap().tensor, sh * HR * 128,
                                    [[128, HR], [1, 128]])
                    nc.gpsimd.dma_gather(
                        out_ap=_ap3(msg[:], [[msg[:].ap[0][0], 128],
                                             [128, ncall], [1, 128]]),
                        in_ap=in_ap,
                        idxs_ap=idxsb[:, c_lo * 8:c_hi * 8],
                        num_idxs=ncall * 128,
                        num_idxs_reg=ncall * 128,
                        elem_size=128)
                    if debug and layer == 1 and ncalls < 8:
                        nc.sync.dma_start(
                            _ap3(dbg["msg1"].ap(),
                                 [[CALLCH * 128, 128], [1, ncall * 128]],
                                 ncalls * 128 * CALLCH * 128),
                            msg[:, :ncall * 128])
                    ncalls += 1
                    for s0 in range(0, ncall, SBATCH):
                        nb = min(SBATCH, ncall - s0)
                        S = spool.tile([128, SBATCH * 256], BF16, tag="S")
                        S3 = _ap3(S[:], [[S[:].ap[0][0], 128], [256, nb], [1, 256]])
                        nc.vector.tensor_tensor(
                            S3, iota_bc(nb),
                            sc_bc(dstloc, c_lo + s0, nb, 256),
                            op=ALU.is_equal)
                        for j in range(nb):
                            ci = c_lo + s0 + j
                            wp = int(chunk_w[ci])
                            wA = 2 * wp
                            g = wA // WGRP
                            if g not in psg:
                                psg[g] = agg_ps.tile(
                                    [128, WGRP * H], F32, tag="agg",
                                    name=f"agg_l{layer}_g{g}")
                            wl = wA - g * WGRP
                            has_b = wA + 1 < W
                            rhs = msg[:, (s0 + j) * 128:(s0 + j) * 128 + H]
                            mm = nc.tensor.matmul(
                                psg[g][:, wl * H:(wl + 1) * H],
                                lhsT=S[:, j * 256:j * 256 + 128],
                                rhs=rhs,
                                start=bool(start_f[ci]),
                                stop=bool(stop_f[ci]) and not has_b,
                                skip_group_check=True)
                            bk = int(meta["bank_of_chunk"][ci])
                            if start_f[ci]:
                                bank_start[bk] = mm
                            elif bk in bank_start:
                                bass._add_dep_helper(
                                    mm.ins, bank_start[bk].ins, sync=False,
                                    reason="psum zero-region order")
                            if has_b:
                                mmb = nc.tensor.matmul(
                                    psg[g][:, (wl + 1) * H:(wl + 2) * H],
                                    lhsT=S[:, j * 256 + 128:(j + 1) * 256],
                                    rhs=rhs,
                                    start=False, stop=bool(stop_f[ci]),
                                    skip_group_check=True)
                                bass._add_dep_helper(
                                    mmb.ins, bank_start[bk].ins, sync=False,
                                    reason="psum zero-region order")
                            if ci in drain_after:
                                gdone = drain_after[ci]
                                drain_group(gdone, psg.pop(gdone), bsb,
                                            layer=layer)
                                if after_group is not None:
                                    after_group(gdone)

            def drain_group(g, ps, bsb, layer=0):
                    wlo = g * WGRP
                    nb = min(WGRP, W - wlo)
                    pstep = ps[:].ap[0][0]
                    if debug and layer == 1:
                        dcp = tpool.tile([128, WGRP * H], F32, tag="dbgcp",
                                         name=f"dbgcp{g}")
                        nc.vector.tensor_copy(dcp[:, :nb * H], ps[:, :nb * H])
                        nc.sync.dma_start(
                            _ap3(dbg["agg1"].ap(), [[W * H, 128], [1, nb * H]],
                                 wlo * H),
                            dcp[:, :nb * H])
                    ps3 = _ap3(ps[:], [[pstep, 128], [H, nb], [1, H]])
                    tmp = tpool.tile([128, WGRP * H], F32, tag="dr")
                    ts = tmp[:].ap[0][0]
                    tmp3 = _ap3(tmp[:], [[ts, 128], [H, nb], [1, H]])
                    to3 = _ap3(t_own[:], [[t_own[:].ap[0][0], 128], [H, nb], [1, H]],
                               t_own[:].offset + wlo * H)
                    # agg + t_own
                    nc.vector.tensor_tensor(tmp3, ps3, to3, op=ALU.add)
                    # * dis
                    nc.vector.tensor_tensor(
                        tmp3, tmp3, sc_bc(disn, wlo, nb, H), op=ALU.mult)
                    # + bias
                    bb = _ap3(bsb[:], [[bsb[:].ap[0][0], 128], [0, nb],
                                       [1, H]], 0)
                    nc.vector.tensor_tensor(tmp3, tmp3, bb, op=ALU.add)
                    # relu -> bf16 node-major
                    hn3 = _ap3(h_nm[:], [[h_nm[:].ap[0][0], 128], [H, nb], [1, H]],
                               h_nm[:].offset + wlo * H)
                    nc.vector.tensor_scalar(
                        hn3, tmp3, 0.0, None, op0=ALU.max)
                    # mask tail-window pads
                    if g == NG - 1 and meta["tailp"] < 128:
                        lastw = W - 1
                        hl = _ap3(h_nm[:], [[h_nm[:].ap[0][0], 128], [1, H]],
                                  h_nm[:].offset + lastw * H)
                        mb = _ap3(mask48[:],
                                  [[mask48[:].ap[0][0], 128], [0, H]], 0)
                        nc.vector.tensor_tensor(hl, hl, mb, op=ALU.mult)

            def to_fm_grp(dst_fm, g):
                for w in range(g * WGRP, min((g + 1) * WGRP, W)):
                    tp = tp_ps.tile([H, 128], BF16, tag="tp")
                    nc.tensor.transpose(
                        out=tp[:],
                        in_=_ap3(h_nm[:], [[h_nm[:].ap[0][0], 128], [1, H]],
                                 h_nm[:].offset + w * H),
                        identity=ident[:])
                    nc.scalar.copy(dst_fm[:, w * 128:(w + 1) * 128], tp[:])

            # ================= layer schedule =================
            with tc.tile_pool(name="xfm", bufs=2) as xpool:
                transform_all_conv1(xpool)
                xo = xpool.tile([F, NPAD], BF16, tag="xfm", name="xo")
                nc.sync.dma_start(xo[:], xown_e.ap())
                transform_own(xo, W1sb, F)
            if stages < 2:
                _finish_stub(nc, tc, tpool, mm_ps, out_e, G, meta)
                return nc
            if debug:
                nc.sync.dma_start(dbg["tbl1"].ap(), tbl.ap())
                nc.sync.dma_start(dbg["town1"].ap(), t_own[:])

            pool_state = {}

            def pooling_grp(g):
                wlo = g * WGRP
                nwg = min(WGRP, W - wlo)
                pps = pool_state["pps"]
                for blk in range(3):
                    for w0 in range(wlo, wlo + nwg, SBATCH):
                        nb = min(SBATCH, wlo + nwg - w0)
                        SG = spool.tile([128, SBATCH * 128], BF16, tag="S")
                        iob = _ap3(iotaP[:], [[iotaP[:].ap[0][0], 128],
                                              [0, nb], [1, 128]], blk * 128)
                        nc.vector.tensor_tensor(
                            _ap3(SG[:], [[SG[:].ap[0][0], 128], [128, nb],
                                         [1, 128]]),
                            iob, sc_bc(glocal, w0, nb, 128), op=ALU.is_equal)
                        for i in range(nb):
                            w = w0 + i
                            mm = nc.tensor.matmul(
                                pps[:, blk * H:(blk + 1) * H],
                                lhsT=SG[:, i * 128:(i + 1) * 128],
                                rhs=_ap3(h_nm[:],
                                         [[h_nm[:].ap[0][0], 128], [1, H]],
                                         h_nm[:].offset + w * H),
                                start=(blk == 0 and w == 0),
                                stop=(blk == 2 and w == W - 1),
                                skip_group_check=True)
                            if blk == 0 and w == 0:
                                pool_state["start"] = mm
                            else:
                                bass._add_dep_helper(
                                    mm.ins, pool_state["start"].ins,
                                    sync=False,
                                    reason="psum zero-region order")

            for l in range(1, 6):
                bsb = b1sb if l == 1 else b2sb
                if l < 5:
                    hf_next = h_fm[(l + 1) % 2]

                    def after_group(g, hf=hf_next):
                        to_fm_grp(hf, g)
                        transform_own_grp(hf, W2sb, g)
                else:
                    pool_state["pps"] = agg_ps.tile(
                        [128, WGRP * H], F32, tag="agg", name="pps")

                    def after_group(g):
                        pooling_grp(g)
                aggregate_and_drain(l, bsb, after_group=after_group)
                if debug:
                    nc.sync.dma_start(dbg[f"h{l}"].ap(), h_nm[:])
                if l < 5:
                    exchange()

            ppsb = tpool.tile([128, PW], F32, tag="ppsb")
            nc.vector.tensor_copy(ppsb[:], pool_state["pps"][:, :PW])
            nc.sync.dma_start(out_e.ap(), ppsb[:])


def _finish_stub(nc, tc, tpool, mm_ps, out_e, G, meta):
    z = tpool.tile([128, 3 * meta["H"]], F32, tag="zstub", name="zstub")
    nc.vector.memset(z[:], 0.0)
    nc.sync.dma_start(out_e.ap(), z[:])


# ---------------------------------------------------------------------------
# Entry points
# ---------------------------------------------------------------------------

def run(inputs, C=8, G=1000, trace=False, stages=99):
    meta, in_maps = prep(
        inputs["x"], inputs["W1"], inputs["b1"], inputs["W2"], inputs["b2"],
        inputs["Wl"], inputs["bl"], inputs["edge_index"], inputs["batch"],
        C=C, G=G)
    nc = bacc.Bacc("TRN2", target_bir_lowering=False, debug=False,
                   num_devices=C)
    build(nc, meta, stages=stages)
    nc.compile()
    from concourse.bass_utils import run_bass_kernel_spmd
    res = run_bass_kernel_spmd(nc, in_maps, core_ids=list(range(C)),
                               trace=trace)
    parts = [res.results[c]["out"] for c in range(C)]
    out = host_finish(meta, parts, inputs, C, G)
    return out, res


def host_finish(meta, parts, inputs, C, G):
    """Combine per-core pooled partial sums, divide by counts, final linear."""
    H = meta["H"]
    pooled = np.zeros(((meta["BLK"] + 3) * 128, H), np.float32)
    for c in range(C):
        part = np.asarray(parts[c], np.float32)   # [128, 3H]
        base = meta["wk"][c] * 128
        for b in range(3):
            pooled[base + b * 128: base + (b + 1) * 128] += \
                part[:, b * H:(b + 1) * H]
    counts = np.bincount(np.asarray(inputs["batch"], np.int64),
                         minlength=G).astype(np.float32)
    pooledG = pooled[:G] / np.maximum(counts, 1.0)[:, None]
    Wl = np.asarray(inputs["Wl"], np.float32).reshape(H, -1)
    bl = np.asarray(inputs["bl"], np.float32)
    return (pooledG @ Wl + bl).astype(np.float32)


def kernel(**inputs):
    out, _ = run(inputs)
    return out



# revision 3
# speedup vs baseline: 2.4778x; 2.4778x over previous
"""GCN (6-layer: conv1 + 4x shared conv2 + mean-pool + linear) on 8 Trainium2
NeuronCores via Bass/Tile.

Strategy (dst-sharded message passing with a replicated gather table):
  - Nodes are sharded contiguously across cores (NPC = N/C per core).
  - Per conv layer: every core transforms its own nodes (h @ W), scales rows
    by dis = deg^-1/2 (GCN norm factorization: enorm = dis[src]*dis[dst], so
    agg[d] = dis[d] * sum_e t[src_e] with t = dis*hw, self term = dis[d]*t[d]),
    and the per-core row slices are AllGather'd into a replicated HBM table.
  - Each core gathers its incident edges' source rows (dma_gather, 256B rows)
    and segment-sums them by destination via one-hot matmuls on the
    TensorEngine (S built on-chip with an iota/is_equal compare, PSUM
    accumulation per 128-node window).
  - Mean-pool partial sums per graph are computed with the same one-hot
    matmul trick, AllGather'd (small), and every core computes the identical
    final linear readout; core 0's output is returned.

The single SPMD program is identical on all cores: all per-core variation
travels through input tensors; chunk/bucket counts are padded to the max
over cores so the instruction stream is uniform.
"""

import math
import sys

sys.path.insert(0, "/opt/trn_rl_repo")

import numpy as np
import ml_dtypes

import concourse.bass as bass
import concourse.mybir as mybir
import concourse.tile as tile
from concourse import bacc
from concourse.masks import make_identity

BF16 = mybir.dt.bfloat16
F32 = mybir.dt.float32
I16 = mybir.dt.int16
ALU = mybir.AluOpType

NP_BF16 = ml_dtypes.bfloat16

CALLCH = 8         # chunks per dma_gather call (1024 idxs = HW packet limit)
SBATCH = 8         # chunks per S-build DVE op
WGRP = 16          # dst windows per PSUM accumulation group


def _ap3(ap, pattern, offset=None):
    """Hand-build a broadcast/strided AP on the same tensor."""
    return bass.AP(ap.tensor, ap.offset if offset is None else offset, pattern)


# ---------------------------------------------------------------------------
# Host preprocessing
# ---------------------------------------------------------------------------

def prep(x, W1, b1, W2, b2, Wl, bl, edge_index, batch, C, G):
    x = np.asarray(x, np.float32)
    W1 = np.asarray(W1, np.float32); b1 = np.asarray(b1, np.float32)
    W2 = np.asarray(W2, np.float32); b2 = np.asarray(b2, np.float32)
    Wl = np.asarray(Wl, np.float32); bl = np.asarray(bl, np.float32)
    edge_index = np.asarray(edge_index, np.int64)
    batch = np.asarray(batch, np.int64)

    N, F = x.shape
    E = edge_index.shape[1]
    H = W1.shape[1]
    assert N % C == 0 and C % 2 == 0
    NPC = N // C
    W = math.ceil(NPC / 128)
    NPAD = W * 128
    ROWS = C * NPAD
    HR = (C // 2) * NPAD
    assert HR <= 32768, HR
    NG = math.ceil(W / WGRP)

    src, dst = edge_index[0], edge_index[1]
    deg = 1.0 + np.bincount(dst, minlength=N).astype(np.float32)
    dis = 1.0 / np.sqrt(deg)

    n = np.arange(N)
    cb = n // NPC
    lp = n % NPC
    p_ = lp % 128
    w_ = lp // 128
    srow = cb * NPAD + p_ * W + w_      # p-major table row of node n
    xcol = cb * NPAD + w_ * 128 + p_    # window-major x_fm column of node n

    # --- edge bucketing -----------------------------------------------------
    ecore = dst // NPC
    edl = dst % NPC
    ew = edl // 128
    ewp = edl // 256            # window-pair (chunks span 2 windows)
    edloc = edl % 256           # dst-local within the pair
    esh = (srow[src] >= HR).astype(np.int64)
    WP = math.ceil(W / 2)
    PPG = WGRP // 2             # pairs per psum group

    cnt = np.zeros((C, 2, WP), np.int64)
    np.add.at(cnt, (ecore, esh, ewp), 1)
    Kb = np.ceil(cnt.max(axis=0) / 128).astype(np.int64)   # [2, WP] chunks
    for wp in range(WP):
        if Kb[:, wp].sum() == 0:
            Kb[0, wp] = 1

    # chunk order: (group, src-half, window-pair)
    chunk_w, chunk_sh = [], []
    seg_bounds = []   # (sh, lo, hi) per (g, sh) segment
    boff = np.zeros((2, WP), np.int64)  # first chunk index of bucket (sh, wp)
    for g in range(NG):
        plo, phi = g * PPG, min((g + 1) * PPG, WP)
        for sh in (0, 1):
            lo = len(chunk_w)
            for wp in range(plo, phi):
                boff[sh, wp] = len(chunk_w)
                for _ in range(int(Kb[sh, wp])):
                    chunk_w.append(wp); chunk_sh.append(sh)
            if len(chunk_w) > lo:
                seg_bounds.append((sh, lo, len(chunk_w)))
    NCHUNK = len(chunk_w)
    chunk_w = np.array(chunk_w); chunk_sh = np.array(chunk_sh)

    # start/stop flags: first/last chunk per PSUM BANK (8 windows of 64 f32
    # = one 2KB zero region; the start bit lazily zeroes the whole bank).
    # Both windows of a pair are always in the same bank.
    bank_of_chunk = (2 * chunk_w) // 8
    start_f = np.zeros(NCHUNK, bool); stop_f = np.zeros(NCHUNK, bool)
    for b in np.unique(bank_of_chunk):
        idxs = np.nonzero(bank_of_chunk == b)[0]
        start_f[idxs[0]] = True; stop_f[idxs[-1]] = True

    # gather calls: slice each segment into <= CALLCH chunks
    calls = []
    for sh, lo, hi in seg_bounds:
        c0 = lo
        while c0 < hi:
            c1 = min(c0 + CALLCH, hi)
            calls.append((sh, c0, c1))
            c0 = c1
    grp_of_chunk = (2 * chunk_w) // WGRP

    # --- per-core edge payloads --------------------------------------------
    idx_all = np.zeros((C, NCHUNK * 128), np.int16)
    dl_all = np.full((C, NCHUNK * 128), 400.0, np.float32)
    for c in range(C):
        m = ecore == c
        es, ish, iw, idl = src[m], esh[m], ewp[m], edloc[m]
        order = np.lexsort((iw, ish))
        es, ish, iw, idl = es[order], ish[order], iw[order], idl[order]
        # rank within bucket
        key = ish * WP + iw
        # edges sorted by key; position = boff[bucket]*128 + rank-in-bucket
        uniq, first = np.unique(key, return_index=True)
        ranks = np.arange(len(key)) - first[np.searchsorted(uniq, key)]
        pos = boff[ish, iw] * 128 + ranks
        idx_all[c, pos] = (srow[es] - ish * HR).astype(np.int16)
        dl_all[c, pos] = idl

    # wrapped-16 index layout, replicated to 128 partitions
    idx16 = np.zeros((C, 128, NCHUNK * 8), np.int16)
    for c in range(C):
        wrapped = idx_all[c].reshape(NCHUNK * 8, 16).T   # [16, NCHUNK*8]
        idx16[c] = np.tile(wrapped, (8, 1))
    dstloc = np.zeros((C, 128, NCHUNK), NP_BF16)
    for c in range(C):
        dstloc[c] = dl_all[c].reshape(NCHUNK, 128).T.astype(NP_BF16)

    # --- node-side tensors --------------------------------------------------
    xfm = np.zeros((F, ROWS), np.float32)
    xfm[:, xcol] = x.T
    xfm = xfm.astype(NP_BF16)

    disALL = np.zeros((128, C * W), np.float32)
    disALL[p_, cb * W + w_] = dis
    disn = np.zeros((C, 128, W), np.float32)
    for c in range(C):
        sl = slice(c * NPC, (c + 1) * NPC)
        disn[c][p_[sl], w_[sl]] = dis[sl]

    tailp = NPC - (W - 1) * 128
    mask48 = (np.arange(128) < tailp).astype(np.float32).reshape(128, 1)

    # --- pooling ------------------------------------------------------------
    BLK = math.ceil(G / 128) + 3
    wk = [int(batch[c * NPC]) // 128 for c in range(C)]
    glocal = np.full((C, 128, W), 1.0e4, np.float32)
    for c in range(C):
        sl = slice(c * NPC, (c + 1) * NPC)
        gl = batch[sl] - 128 * wk[c]
        assert gl.min() >= 0 and gl.max() < 384, (c, gl.min(), gl.max())
        glocal[c][p_[sl], w_[sl]] = gl
    counts = np.bincount(batch, minlength=G).astype(np.float32)
    invc = np.ones((128, BLK), np.float32)
    gg = np.arange(G)
    invc[gg % 128, gg // 128] = 1.0 / np.maximum(counts, 1.0)

    meta = dict(
        N=N, F=F, H=H, E=E, G=G, C=C, NPC=NPC, W=W, NPAD=NPAD, ROWS=ROWS,
        HR=HR, NG=NG, NCHUNK=NCHUNK, chunk_w=chunk_w, chunk_sh=chunk_sh,
        bank_of_chunk=bank_of_chunk,
        start_f=start_f, stop_f=stop_f, calls=calls, grp_of_chunk=grp_of_chunk,
        wk=wk, BLK=BLK, blv=float(bl.reshape(-1)[0]), tailp=tailp,
    )

    shared = dict(
        xfm=xfm,
        disALL=disALL,
        W1sb=W1.astype(NP_BF16),
        W2sb=W2.astype(NP_BF16),
        Wlsb=Wl.astype(NP_BF16),
        b1sb=np.tile(b1.reshape(1, H), (128, 1)).astype(np.float32),
        b2sb=np.tile(b2.reshape(1, H), (128, 1)).astype(np.float32),
        iota128=np.tile(np.arange(256, dtype=np.float32).reshape(1, 256),
                        (128, 1)).astype(NP_BF16),
        iotaP=np.tile(np.arange(384, dtype=np.float32).reshape(1, 384),
                      (128, 1)),
        invc=invc,
        mask48=mask48,
    )
    in_maps = []
    for c in range(C):
        m = dict(shared)
        m["xown"] = np.ascontiguousarray(
            xfm[:, c * NPAD:(c + 1) * NPAD])
        m["idx16"] = idx16[c]
        m["dstloc"] = dstloc[c]
        m["disn"] = disn[c]
        m["glocal"] = glocal[c]
        in_maps.append(m)
    return meta, in_maps


# ---------------------------------------------------------------------------
# Bass/Tile program
# ---------------------------------------------------------------------------

def build(nc, meta, debug=False, stages=99):
    F, H, C = meta["F"], meta["H"], meta["C"]
    W, NPAD, ROWS, HR = meta["W"], meta["NPAD"], meta["ROWS"], meta["HR"]
    NG, NCHUNK, BLK, G = meta["NG"], meta["NCHUNK"], meta["BLK"], meta["G"]
    chunk_w, chunk_sh = meta["chunk_w"], meta["chunk_sh"]
    start_f, stop_f = meta["start_f"], meta["stop_f"]
    calls, wk = meta["calls"], meta["wk"]
    rg = [list(range(C))]

    # external inputs
    xfm_e = nc.dram_tensor("xfm", [F, ROWS], BF16, kind="ExternalInput")
    disALL_e = nc.dram_tensor("disALL", [128, C * W], F32, kind="ExternalInput")
    W1_e = nc.dram_tensor("W1sb", [F, H], BF16, kind="ExternalInput")
    W2_e = nc.dram_tensor("W2sb", [H, H], BF16, kind="ExternalInput")
    Wl_e = nc.dram_tensor("Wlsb", [H, 1], BF16, kind="ExternalInput")
    b1_e = nc.dram_tensor("b1sb", [128, H], F32, kind="ExternalInput")
    b2_e = nc.dram_tensor("b2sb", [128, H], F32, kind="ExternalInput")
    iota_e = nc.dram_tensor("iota128", [128, 256], BF16, kind="ExternalInput")
    iotaP_e = nc.dram_tensor("iotaP", [128, 384], F32, kind="ExternalInput")
    invc_e = nc.dram_tensor("invc", [128, BLK], F32, kind="ExternalInput")
    mask_e = nc.dram_tensor("mask48", [128, 1], F32, kind="ExternalInput")
    xown_e = nc.dram_tensor("xown", [F, NPAD], BF16, kind="ExternalInput")
    idx_e = nc.dram_tensor("idx16", [128, NCHUNK * 8], I16, kind="ExternalInput")
    dstloc_e = nc.dram_tensor("dstloc", [128, NCHUNK], BF16, kind="ExternalInput")
    disn_e = nc.dram_tensor("disn", [128, W], F32, kind="ExternalInput")
    glocal_e = nc.dram_tensor("glocal", [128, W], F32, kind="ExternalInput")
    out_e = nc.dram_tensor("out", [128, 3 * H], F32, kind="ExternalOutput")
    dbg = {}
    if debug:
        W_ = W; H_ = H
        dbg["tbl1"] = nc.dram_tensor("dbg_tbl1", [ROWS, 128], BF16,
                                     kind="ExternalOutput")
        dbg["town1"] = nc.dram_tensor("dbg_town1", [128, W_ * H_], BF16,
                                      kind="ExternalOutput")
        for l in range(1, 6):
            dbg[f"h{l}"] = nc.dram_tensor(f"dbg_h{l}", [128, W_ * H_], BF16,
                                          kind="ExternalOutput")
        dbg["agg1"] = nc.dram_tensor("dbg_agg1", [128, W_ * H_], F32,
                                     kind="ExternalOutput")
        dbg["msg1"] = nc.dram_tensor("dbg_msg1", [8, 128, CALLCH * 128], BF16,
                                     kind="ExternalOutput")
        dbg["S1"] = nc.dram_tensor("dbg_S1", [40, 128, SBATCH * 128], BF16,
                                   kind="ExternalOutput")
        dbg["poolf"] = nc.dram_tensor("dbg_poolf", [128, BLK * H_], F32,
                                      kind="ExternalOutput")

    # internal DRAM
    PW = 3 * H
    shared = "Shared" if C > 4 else "Local"
    tbl = nc.dram_tensor("tbl", [ROWS, 128], BF16, addr_space=shared)
    own_slice = nc.dram_tensor("own_slice", [NPAD, 128], BF16)
    ppart = nc.dram_tensor("ppart", [128, PW], BF16)
    pall = nc.dram_tensor("pall", [C, 128, PW], BF16, addr_space=shared)

    with tile.TileContext(nc) as tc:
        from contextlib import ExitStack
        with ExitStack() as ctx:
            cpool = ctx.enter_context(tc.tile_pool(name="const", bufs=1))
            spool = ctx.enter_context(tc.tile_pool(name="s", bufs=8))
            mpool = ctx.enter_context(tc.tile_pool(name="msg", bufs=8))
            hpool = ctx.enter_context(tc.tile_pool(name="h", bufs=1))
            tpool = ctx.enter_context(tc.tile_pool(name="tmp", bufs=3))
            agg_ps = ctx.enter_context(
                tc.tile_pool(name="aggps", bufs=2, space="PSUM"))
            mm_ps = ctx.enter_context(
                tc.tile_pool(name="mmps", bufs=2, space="PSUM"))
            tp_ps = ctx.enter_context(
                tc.tile_pool(name="tpps", bufs=2, space="PSUM"))

            # ---- load constants into SBUF ----
            def load(name, ext, shape, dt):
                t = cpool.tile(shape, dt, tag=name)
                nc.sync.dma_start(t[:], ext.ap())
                return t

            W1sb = load("W1", W1_e, [F, H], BF16)
            W2sb = load("W2", W2_e, [H, H], BF16)
            Wlsb = load("Wl", Wl_e, [H, 1], BF16)
            b1sb = load("b1", b1_e, [128, H], F32)
            b2sb = load("b2", b2_e, [128, H], F32)
            iota = load("iota", iota_e, [128, 256], BF16)
            iotaP = load("iotaP", iotaP_e, [128, 384], F32)
            invc = load("invc", invc_e, [128, BLK], F32)
            mask48 = load("mask48", mask_e, [128, 1], F32)
            idxsb = load("idx", idx_e, [128, NCHUNK * 8], I16)
            dstloc = load("dstloc", dstloc_e, [128, NCHUNK], BF16)
            disn = load("disn", disn_e, [128, W], F32)
            disALL = load("disALL", disALL_e, [128, C * W], F32)
            glocal = load("glocal", glocal_e, [128, W], F32)
            ident = cpool.tile([128, 128], BF16, tag="ident")
            make_identity(nc, ident[:])
            identF = cpool.tile([128, 128], F32, tag="identF")
            make_identity(nc, identF[:])

            # persistent per-layer state tiles
            h_fm = [cpool.tile([H, NPAD], BF16, tag=f"hfm{i}", name=f"hfm{i}")
                    for i in range(2)]
            t_own = cpool.tile([128, W * H], BF16, tag="town")
            tpad = cpool.tile([128, W * 128], BF16, tag="tpad")
            nc.vector.memset(tpad[:], 0.0)
            h_nm = cpool.tile([128, W * H], BF16, tag="hnm")

            def iota_bc(nb, width=256, base=0):
                a = iota[:]
                return _ap3(a, [[a.ap[0][0], 128], [0, nb], [1, width]], base)

            def sc_bc(t, lo, n, inner, pitch=None):
                a = t[:]
                step = a.ap[0][0]
                return _ap3(a, [[step, 128], [1, n], [0, inner]], a.offset + lo)

            # ---------------- conv transform helpers ----------------
            def transform_own_grp(h_src, Wsb, g):
                """own-node transform + t_own (dis*hw) + padded copy, windows
                of group g only."""
                wlo = g * WGRP
                nb = min(WGRP, W - wlo)
                ps = agg_ps.tile([128, WGRP * H], F32, tag="agg",
                                 name=f"tf{g}")
                for i in range(nb):
                    w = wlo + i
                    nc.tensor.matmul(
                        ps[:, i * H:(i + 1) * H],
                        lhsT=h_src[:, w * 128:(w + 1) * 128],
                        rhs=Wsb[:],
                        start=True, stop=True, skip_group_check=True)
                ps3 = _ap3(ps[:], [[ps[:].ap[0][0], 128], [H, nb], [1, H]])
                t3 = _ap3(t_own[:], [[t_own[:].ap[0][0], 128], [H, nb], [1, H]],
                          t_own[:].offset + wlo * H)
                nc.vector.tensor_tensor(
                    t3, ps3, sc_bc(disn, wlo, nb, H), op=ALU.mult)
                # spread compact -> padded (pad half is stale junk, never read)
                tp3 = _ap3(tpad[:], [[tpad[:].ap[0][0], 128], [128, nb], [1, H]],
                           tpad[:].offset + wlo * 128)
                to3 = _ap3(t_own[:], [[t_own[:].ap[0][0], 128], [H, nb], [1, H]],
                           t_own[:].offset + wlo * H)
                nc.vector.tensor_copy(tp3, to3)

            def transform_own(h_src, Wsb, K=None):
                for g in range(NG):
                    transform_own_grp(h_src, Wsb, g)

            def transform_all_conv1(xpool):
                """conv1: full-graph transform, writes the whole table."""
                NBAT = math.ceil(W / WGRP)
                for cblk in range(C):
                    xblk = xpool.tile([F, NPAD], BF16, tag="xfm")
                    nc.sync.dma_start(
                        xblk[:], xfm_e.ap()[:, cblk * NPAD:(cblk + 1) * NPAD])
                    for b in range(NBAT):
                        wlo = b * WGRP
                        nb = min(WGRP, W - wlo)
                        ps = agg_ps.tile([128, WGRP * H], F32, tag="agg")
                        for i in range(nb):
                            col = (wlo + i) * 128
                            nc.tensor.matmul(
                                ps[:, i * H:(i + 1) * H],
                                lhsT=xblk[:, col:col + 128],
                                rhs=W1sb[:],
                                start=True, stop=True, skip_group_check=True)
                        pd = tpool.tile([128, WGRP * 128], BF16, tag="c1pad")
                        nc.vector.memset(pd[:], 0.0)
                        ps3 = _ap3(ps[:], [[ps[:].ap[0][0], 128], [H, nb], [1, H]])
                        pd3 = _ap3(pd[:], [[pd[:].ap[0][0], 128], [128, nb], [1, H]])
                        nc.vector.tensor_tensor(
                            pd3, ps3, sc_bc(disALL, cblk * W + wlo, nb, H),
                            op=ALU.mult)
                        # rows cblk*NPAD + p*W + (wlo..wlo+nb)
                        dr = _ap3(tbl.ap(),
                                  [[W * 128, 128], [128, nb], [1, 128]],
                                  (cblk * NPAD + wlo) * 128)
                        nc.sync.dma_start(dr, pd[:, :nb * 128])

            def exchange():
                # padded own slice -> own_slice dram (contiguous), AllGather
                nc.sync.dma_start(
                    _ap3(own_slice.ap(), [[W * 128, 128], [1, W * 128]], 0),
                    tpad[:])
                nc.gpsimd.collective_compute(
                    "AllGather", ALU.bypass, replica_groups=rg,
                    ins=[own_slice.ap().opt()], outs=[tbl.ap().opt()])

            # ---------------- aggregation (+ per-group drain) ----------------
            def aggregate_and_drain(layer, bsb, after_group=None):
                ncalls = 0
                nsb = 0
                psg = {}
                bank_start = {}
                glast = {}
                for ci in range(NCHUNK):
                    glast[int(meta["grp_of_chunk"][ci])] = ci
                drain_after = {v: k for k, v in glast.items()}
                for (sh, c_lo, c_hi) in calls:
                    ncall = c_hi - c_lo
                    msg = mpool.tile([128, CALLCH * 128], BF16, tag="msg")
                    in_ap = bass.AP(tbl.ap().tensor, sh * HR * 128,
                                    [[128, HR], [1, 128]])
                    nc.gpsimd.dma_gather(
                        out_ap=_ap3(msg[:], [[msg[:].ap[0][0], 128],
                                             [128, ncall], [1, 128]]),
                        in_ap=in_ap,
                        idxs_ap=idxsb[:, c_lo * 8:c_hi * 8],
                        num_idxs=ncall * 128,
                        num_idxs_reg=ncall * 128,
                        elem_size=128,
                        queue_num=ncalls % 4)
                    if debug and layer == 1 and ncalls < 8:
                        nc.sync.dma_start(
                            _ap3(dbg["msg1"].ap(),
                                 [[CALLCH * 128, 128], [1, ncall * 128]],
                                 ncalls * 128 * CALLCH * 128),
                            msg[:, :ncall * 128])
                    ncalls += 1
                    for s0 in range(0, ncall, SBATCH):
                        nb = min(SBATCH, ncall - s0)
                        S = spool.tile([128, SBATCH * 256], BF16, tag="S")
                        S3 = _ap3(S[:], [[S[:].ap[0][0], 128], [256, nb], [1, 256]])
                        nc.vector.tensor_tensor(
                            S3, iota_bc(nb),
                            sc_bc(dstloc, c_lo + s0, nb, 256),
                            op=ALU.is_equal)
                        for j in range(nb):
                            ci = c_lo + s0 + j
                            wp = int(chunk_w[ci])
                            wA = 2 * wp
                            g = wA // WGRP
                            if g not in psg:
                                psg[g] = agg_ps.tile(
                                    [128, WGRP * H], F32, tag="agg",
                                    name=f"agg_l{layer}_g{g}")
                            wl = wA - g * WGRP
                            has_b = wA + 1 < W
                            rhs = msg[:, (s0 + j) * 128:(s0 + j) * 128 + H]
                            mm = nc.tensor.matmul(
                                psg[g][:, wl * H:(wl + 1) * H],
                                lhsT=S[:, j * 256:j * 256 + 128],
                                rhs=rhs,
                                start=bool(start_f[ci]),
                                stop=bool(stop_f[ci]) and not has_b,
                                skip_group_check=True)
                            bk = int(meta["bank_of_chunk"][ci])
                            if start_f[ci]:
                                bank_start[bk] = mm
                            elif bk in bank_start:
                                bass._add_dep_helper(
                                    mm.ins, bank_start[bk].ins, sync=False,
                                    reason="psum zero-region order")
                            if has_b:
                                mmb = nc.tensor.matmul(
                                    psg[g][:, (wl + 1) * H:(wl + 2) * H],
                                    lhsT=S[:, j * 256 + 128:(j + 1) * 256],
                                    rhs=rhs,
                                    start=False, stop=bool(stop_f[ci]),
                                    skip_group_check=True)
                                bass._add_dep_helper(
                                    mmb.ins, bank_start[bk].ins, sync=False,
                                    reason="psum zero-region order")
                            if ci in drain_after:
                                gdone = drain_after[ci]
                                drain_group(gdone, psg.pop(gdone), bsb,
                                            layer=layer)
                                if after_group is not None:
                                    after_group(gdone)

            def drain_group(g, ps, bsb, layer=0):
                    wlo = g * WGRP
                    nb = min(WGRP, W - wlo)
                    pstep = ps[:].ap[0][0]
                    if debug and layer == 1:
                        dcp = tpool.tile([128, WGRP * H], F32, tag="dbgcp",
                                         name=f"dbgcp{g}")
                        nc.vector.tensor_copy(dcp[:, :nb * H], ps[:, :nb * H])
                        nc.sync.dma_start(
                            _ap3(dbg["agg1"].ap(), [[W * H, 128], [1, nb * H]],
                                 wlo * H),
                            dcp[:, :nb * H])
                    ps3 = _ap3(ps[:], [[pstep, 128], [H, nb], [1, H]])
                    tmp = tpool.tile([128, WGRP * H], F32, tag="dr")
                    ts = tmp[:].ap[0][0]
                    tmp3 = _ap3(tmp[:], [[ts, 128], [H, nb], [1, H]])
                    to3 = _ap3(t_own[:], [[t_own[:].ap[0][0], 128], [H, nb], [1, H]],
                               t_own[:].offset + wlo * H)
                    # agg + t_own
                    nc.vector.tensor_tensor(tmp3, ps3, to3, op=ALU.add)
                    # * dis
                    nc.vector.tensor_tensor(
                        tmp3, tmp3, sc_bc(disn, wlo, nb, H), op=ALU.mult)
                    # + bias
                    bb = _ap3(bsb[:], [[bsb[:].ap[0][0], 128], [0, nb],
                                       [1, H]], 0)
                    nc.vector.tensor_tensor(tmp3, tmp3, bb, op=ALU.add)
                    # relu -> bf16 node-major
                    hn3 = _ap3(h_nm[:], [[h_nm[:].ap[0][0], 128], [H, nb], [1, H]],
                               h_nm[:].offset + wlo * H)
                    nc.vector.tensor_scalar(
                        hn3, tmp3, 0.0, None, op0=ALU.max)
                    # mask tail-window pads
                    if g == NG - 1 and meta["tailp"] < 128:
                        lastw = W - 1
                        hl = _ap3(h_nm[:], [[h_nm[:].ap[0][0], 128], [1, H]],
                                  h_nm[:].offset + lastw * H)
                        mb = _ap3(mask48[:],
                                  [[mask48[:].ap[0][0], 128], [0, H]], 0)
                        nc.vector.tensor_tensor(hl, hl, mb, op=ALU.mult)

            def to_fm_grp(dst_fm, g):
                for w in range(g * WGRP, min((g + 1) * WGRP, W)):
                    tp = tp_ps.tile([H, 128], BF16, tag="tp")
                    nc.tensor.transpose(
                        out=tp[:],
                        in_=_ap3(h_nm[:], [[h_nm[:].ap[0][0], 128], [1, H]],
                                 h_nm[:].offset + w * H),
                        identity=ident[:])
                    nc.scalar.copy(dst_fm[:, w * 128:(w + 1) * 128], tp[:])

            # ================= layer schedule =================
            with tc.tile_pool(name="xfm", bufs=2) as xpool:
                transform_all_conv1(xpool)
                xo = xpool.tile([F, NPAD], BF16, tag="xfm", name="xo")
                nc.sync.dma_start(xo[:], xown_e.ap())
                transform_own(xo, W1sb, F)
            if stages < 2:
                _finish_stub(nc, tc, tpool, mm_ps, out_e, G, meta)
                return nc
            if debug:
                nc.sync.dma_start(dbg["tbl1"].ap(), tbl.ap())
                nc.sync.dma_start(dbg["town1"].ap(), t_own[:])

            pool_state = {}

            def pooling_grp(g):
                wlo = g * WGRP
                nwg = min(WGRP, W - wlo)
                pps = pool_state["pps"]
                for blk in range(3):
                    for w0 in range(wlo, wlo + nwg, SBATCH):
                        nb = min(SBATCH, wlo + nwg - w0)
                        SG = spool.tile([128, SBATCH * 128], BF16, tag="S")
                        iob = _ap3(iotaP[:], [[iotaP[:].ap[0][0], 128],
                                              [0, nb], [1, 128]], blk * 128)
                        nc.vector.tensor_tensor(
                            _ap3(SG[:], [[SG[:].ap[0][0], 128], [128, nb],
                                         [1, 128]]),
                            iob, sc_bc(glocal, w0, nb, 128), op=ALU.is_equal)
                        for i in range(nb):
                            w = w0 + i
                            mm = nc.tensor.matmul(
                                pps[:, blk * H:(blk + 1) * H],
                                lhsT=SG[:, i * 128:(i + 1) * 128],
                                rhs=_ap3(h_nm[:],
                                         [[h_nm[:].ap[0][0], 128], [1, H]],
                                         h_nm[:].offset + w * H),
                                start=(blk == 0 and w == 0),
                                stop=(blk == 2 and w == W - 1),
                                skip_group_check=True)
                            if blk == 0 and w == 0:
                                pool_state["start"] = mm
                            else:
                                bass._add_dep_helper(
                                    mm.ins, pool_state["start"].ins,
                                    sync=False,
                                    reason="psum zero-region order")

            for l in range(1, 6):
                bsb = b1sb if l == 1 else b2sb
                if l < 5:
                    hf_next = h_fm[(l + 1) % 2]

                    def after_group(g, hf=hf_next):
                        to_fm_grp(hf, g)
                        transform_own_grp(hf, W2sb, g)
                else:
                    pool_state["pps"] = agg_ps.tile(
                        [128, WGRP * H], F32, tag="agg", name="pps")

                    def after_group(g):
                        pooling_grp(g)
                aggregate_and_drain(l, bsb, after_group=after_group)
                if debug:
                    nc.sync.dma_start(dbg[f"h{l}"].ap(), h_nm[:])
                if l < 5:
                    exchange()

            ppsb = tpool.tile([128, PW], F32, tag="ppsb")
            nc.vector.tensor_copy(ppsb[:], pool_state["pps"][:, :PW])
            nc.sync.dma_start(out_e.ap(), ppsb[:])


def _finish_stub(nc, tc, tpool, mm_ps, out_e, G, meta):
    z = tpool.tile([128, 3 * meta["H"]], F32, tag="zstub", name="zstub")
    nc.vector.memset(z[:], 0.0)
    nc.sync.dma_start(out_e.ap(), z[:])


# ---------------------------------------------------------------------------
# Entry points
# ---------------------------------------------------------------------------

def run(inputs, C=8, G=1000, trace=False, stages=99):
    meta, in_maps = prep(
        inputs["x"], inputs["W1"], inputs["b1"], inputs["W2"], inputs["b2"],
        inputs["Wl"], inputs["bl"], inputs["edge_index"], inputs["batch"],
        C=C, G=G)
    nc = bacc.Bacc("TRN2", target_bir_lowering=False, debug=False,
                   num_devices=C, num_swdge_queues=4)
    build(nc, meta, stages=stages)
    nc.compile()
    from concourse.bass_utils import run_bass_kernel_spmd
    res = run_bass_kernel_spmd(nc, in_maps, core_ids=list(range(C)),
                               trace=trace)
    parts = [res.results[c]["out"] for c in range(C)]
    out = host_finish(meta, parts, inputs, C, G)
    return out, res


def host_finish(meta, parts, inputs, C, G):
    """Combine per-core pooled partial sums, divide by counts, final linear."""
    H = meta["H"]
    pooled = np.zeros(((meta["BLK"] + 3) * 128, H), np.float32)
    for c in range(C):
        part = np.asarray(parts[c], np.float32)   # [128, 3H]
        base = meta["wk"][c] * 128
        for b in range(3):
            pooled[base + b * 128: base + (b + 1) * 128] += \
                part[:, b * H:(b + 1) * H]
    counts = np.bincount(np.asarray(inputs["batch"], np.int64),
                         minlength=G).astype(np.float32)
    pooledG = pooled[:G] / np.maximum(counts, 1.0)[:, None]
    Wl = np.asarray(inputs["Wl"], np.float32).reshape(H, -1)
    bl = np.asarray(inputs["bl"], np.float32)
    return (pooledG @ Wl + bl).astype(np.float32)


def kernel(**inputs):
    out, _ = run(inputs)
    return out

